# revision 2
# baseline (speedup 1.0000x reference)
"""LMClassifier forward (mean masked cross-entropy) on 8 Trainium2 cores.

Valid-token packing: only positions t < lens[b]-2 contribute to the mean
NLL, so the host packs just those ~48% of (t, b) pairs into dense token
columns before sharding.  Sharding: 4 token-groups x 2 vocab-groups.
Each core computes
  emb = sigmoid(ctx @ W1.T + b1)            (its packed tokens, all E)
  sumexp[tok] = sum_v exp((emb @ W2s.T) * inv_temp)   (its vocab shard)
  tgt_raw[tok] = emb . W2[tgt[tok]]          (ones-matmul partition reduce)
Host combines sumexp across vocab shards (logits are O(1), so no
max-subtraction is needed) and assembles the mean NLL over valid tokens.
"""

import contextlib
import math

import numpy as np
import ml_dtypes

import concourse.bacc as bacc
import concourse.tile as tile
import concourse.mybir as mybir
from concourse.bass_utils import run_bass_kernel_spmd

BF16 = mybir.dt.bfloat16
FP32 = mybir.dt.float32
AF = mybir.ActivationFunctionType


FP8 = mybir.dt.float8e4
FP8NP = mybir.dt.np(mybir.dt.float8e4)
W2_SCALE = 64.0  # keeps fp8-cast W2 out of the denormal range
W1_SCALE = 64.0  # same for W1; sigmoid's free affine divides it back out


class Cfg:
    def __init__(self, H, E, NTA, NTB, VC, inv_temp=1.0, use_b2=False):
        assert H % 128 == 0 and E % 128 == 0
        assert NTA % 512 == 0 and NTB % 128 == 0 and NTB <= NTA
        assert VC % 128 == 0
        self.H, self.E, self.NTA, self.NTB, self.VC = H, E, NTA, NTB, VC
        self.inv_temp = float(inv_temp)
        self.use_b2 = use_b2
        self.n_k = H // 128     # contraction tiles for matmul1
        self.n_e = E // 128     # e-blocks (also contraction tiles for matmul2)
        self.n_s = NTA // 512   # token superblocks (phase A)
        self.n_sub = NTB // 128 # token subblocks (phase B)
        # vocab blocks: 1024-wide plus one optional 128*k tail
        self.vblk = [1024] * (VC // 1024)
        if VC % 1024:
            self.vblk.append(VC % 1024)
        self.n_vp = len(self.vblk)
        assert self.n_e % 2 == 0 and self.n_k % 2 == 0


def build_lm_program(cfg):
    """Build the per-core SPMD Bass program. Returns compiled nc."""
    H, E, NTA, NTB, VC = cfg.H, cfg.E, cfg.NTA, cfg.NTB, cfg.VC
    nc = bacc.Bacc("TRN2", debug=False, target_bir_lowering=False)

    ctxT = nc.dram_tensor("ctxT", [H, NTA], FP8, kind="ExternalInput").ap()
    w1t = nc.dram_tensor("w1t", [H, E], FP8, kind="ExternalInput").ap()
    b1 = nc.dram_tensor("b1", [E, 1], FP32, kind="ExternalInput").ap()
    w2t = nc.dram_tensor("w2t", [E, VC], FP8, kind="ExternalInput").ap()
    w2tgtT = nc.dram_tensor("w2tgtT", [E, NTA], BF16, kind="ExternalInput").ap()
    ones_in = nc.dram_tensor("ones_in", [128, 1], BF16, kind="ExternalInput").ap()
    if cfg.use_b2:
        b2row = nc.dram_tensor("b2row", [1, VC], FP32, kind="ExternalInput").ap()
    sumexp_out = nc.dram_tensor(
        "sumexp_out", [128, cfg.n_sub], FP32, kind="ExternalOutput"
    ).ap()
    tgt_out = nc.dram_tensor("tgt_out", [1, NTA], FP32, kind="ExternalOutput").ap()

    voff = np.cumsum([0] + cfg.vblk)  # vocab block column offsets

    with contextlib.ExitStack() as ex:
        tc = ex.enter_context(tile.TileContext(nc))
        # persistent sbuf tensors
        const_pool = ex.enter_context(tc.tile_pool(name="const", bufs=1))
        w1_pool = ex.enter_context(tc.tile_pool(name="w1", bufs=1))
        emb_pool = ex.enter_context(tc.tile_pool(name="emb", bufs=1))
        acc_pool = ex.enter_context(tc.tile_pool(name="acc", bufs=1))
        # streamed tiles
        ctx_pool = ex.enter_context(tc.tile_pool(name="ctx", bufs=2))
        w2_pool = ex.enter_context(tc.tile_pool(name="w2", bufs=2))
        tgtw_pool = ex.enter_context(tc.tile_pool(name="tgtw", bufs=2))
        tmp_pool = ex.enter_context(tc.tile_pool(name="tmp", bufs=2))
        ps1_pool = ex.enter_context(tc.tile_pool(name="ps1", bufs=2, space="PSUM"))
        ps2_pool = ex.enter_context(tc.tile_pool(name="ps2", bufs=2, space="PSUM"))
        pst_pool = ex.enter_context(tc.tile_pool(name="pst", bufs=2, space="PSUM"))

        # ---- constants ----
        W1S = w1_pool.tile([128, cfg.n_k, E], FP8, tag="w1s")
        for k in range(cfg.n_k):
            eng = nc.sync if k % 2 == 0 else nc.scalar
            eng.dma_start(W1S[:, k : k + 1, :], w1t[k * 128 : (k + 1) * 128, :])
        B1S = const_pool.tile([128, cfg.n_e], FP32, tag="b1s")
        nc.sync.dma_start(B1S[:, :], b1.rearrange("(e p) one -> p (e one)", p=128))
        ONES = const_pool.tile([128, 1], BF16, tag="ones")
        nc.sync.dma_start(ONES[:, :], ones_in[:, :])
        if cfg.use_b2:
            B2S = const_pool.tile([1, VC], FP32, tag="b2s")
            nc.sync.dma_start(B2S[:, :], b2row[:, :])
            ONE1 = const_pool.tile([1, 128], FP32, tag="one1")
            nc.any.memset(ONE1[:, :], 1.0)

        EMB = emb_pool.tile([128, cfg.n_s * cfg.n_e * 512], BF16, tag="emb")
        EMB8 = emb_pool.tile([128, cfg.n_e, NTA], FP8, tag="emb8")
        SUMP = acc_pool.tile([128, cfg.n_sub * cfg.n_vp], FP32, tag="sump")
        SOUT = acc_pool.tile([128, cfg.n_sub], FP32, tag="sout")
        TGT = acc_pool.tile([1, NTA], FP32, tag="tgt")

        # ---- phase A: emb = sigmoid(W1 @ ctx + b1), [e, t] layout ----
        sig_scale = 1.0 / W1_SCALE
        w2_prefetch = {}
        with nc.named_scope("phaseA"):
            for s in range(cfg.n_s):
                # stream the first W2 shard tile(s) in under phase-A compute so
                # phase B starts without a DMA stall
                if s >= 1 and (s - 1) not in w2_prefetch and (s - 1) < cfg.n_vp - 1:
                    vp = s - 1
                    W2P = w2_pool.tile([128, cfg.n_e, 1024], FP8, tag="w2s")
                    for e in range(cfg.n_e):
                        nc.sync.dma_start(
                            W2P[:, e : e + 1, : cfg.vblk[vp]],
                            w2t[e * 128 : (e + 1) * 128, voff[vp] : voff[vp + 1]],
                        )
                    w2_prefetch[vp] = W2P
                CTXS = ctx_pool.tile([128, cfg.n_k, 512], FP8, tag="ctxs")
                for k in range(cfg.n_k):
                    eng = nc.scalar if (s == 0 and k % 2 == 0) else nc.sync
                    eng.dma_start(
                        CTXS[:, k : k + 1, :],
                        ctxT[k * 128 : (k + 1) * 128, s * 512 : (s + 1) * 512],
                    )
                for e in range(cfg.n_e):
                    ps1 = ps1_pool.tile([128, 512], FP32, tag="ps1")
                    for kp in range(cfg.n_k // 2):
                        nc.tensor.matmul(
                            ps1[:, :],
                            W1S[:, 2 * kp : 2 * kp + 2, e * 128 : (e + 1) * 128],
                            CTXS[:, 2 * kp : 2 * kp + 2, :],
                            start=(kp == 0),
                            stop=(kp == cfg.n_k // 2 - 1),
                            perf_mode=mybir.MatmulPerfMode.DoubleRow,
                        )
                    nc.scalar.activation(
                        EMB[:, (s * cfg.n_e + e) * 512 : (s * cfg.n_e + e + 1) * 512],
                        ps1[:, :],
                        AF.Sigmoid,
                        bias=B1S[:, e : e + 1],
                        scale=sig_scale,
                    )
                    nc.scalar.activation(
                        EMB8[:, e : e + 1, s * 512 : (s + 1) * 512],
                        ps1[:, :],
                        AF.Sigmoid,
                        bias=B1S[:, e : e + 1],
                        scale=sig_scale,
                    )

                # ---- phase A2: tgt_raw for this superblock ----
                TGW = tgtw_pool.tile([128, cfg.n_e * 512], BF16, tag="tgw")
                nc.sync.dma_start(
                    TGW[:, :],
                    w2tgtT.rearrange("(e p) t -> p e t", p=128)[
                        :, :, s * 512 : (s + 1) * 512
                    ],
                )
                pst = pst_pool.tile([1, 512], FP32, tag="pst")
                for e in range(cfg.n_e):
                    tmp = tmp_pool.tile([128, 512], BF16, tag="tmp")
                    nc.vector.tensor_mul(
                        tmp[:, :],
                        EMB[:, (s * cfg.n_e + e) * 512 : (s * cfg.n_e + e + 1) * 512],
                        TGW[:, e * 512 : (e + 1) * 512],
                    )
                    nc.tensor.matmul(
                        pst[:, :],
                        ONES[:, :],
                        tmp[:, :],
                        start=(e == 0),
                        stop=(e == cfg.n_e - 1),
                    )
                nc.vector.tensor_copy(TGT[:, s * 512 : (s + 1) * 512], pst[:, :])

        # ---- phase B: logits, exp, accumulate ----
        exp_scale = cfg.inv_temp / W2_SCALE
        with nc.named_scope("phaseB"):
            for vp in range(cfg.n_vp):
                W = cfg.vblk[vp]
                if vp in w2_prefetch:
                    W2S8 = w2_prefetch.pop(vp)
                else:
                    W2S8 = w2_pool.tile([128, cfg.n_e, 1024], FP8, tag="w2s")
                    for e in range(cfg.n_e):
                        nc.sync.dma_start(
                            W2S8[:, e : e + 1, :W],
                            w2t[e * 128 : (e + 1) * 128, voff[vp] : voff[vp + 1]],
                        )
                hblk = []
                h0 = 0
                while h0 < W:
                    hblk.append((h0, min(512, W - h0)))
                    h0 += 512
                for sub in range(cfg.n_sub):
                    ps2 = ps2_pool.tile([128, 1024], FP32, tag="ps2")
                    for ep in range(cfg.n_e // 2):
                        lhsT = EMB8[
                            :, 2 * ep : 2 * ep + 2, sub * 128 : (sub + 1) * 128
                        ]
                        for h0, hw in hblk:
                            nc.tensor.matmul(
                                ps2[:, h0 : h0 + hw],
                                lhsT,
                                W2S8[:, 2 * ep : 2 * ep + 2, h0 : h0 + hw],
                                start=(ep == 0),
                                stop=(ep == cfg.n_e // 2 - 1) and not cfg.use_b2,
                                perf_mode=mybir.MatmulPerfMode.DoubleRow,
                            )
                    if cfg.use_b2:
                        for h0, hw in hblk:
                            nc.tensor.matmul(
                                ps2[:, h0 : h0 + hw],
                                ONE1[:, :],
                                B2S[:, voff[vp] + h0 : voff[vp] + h0 + hw],
                                start=False,
                                stop=True,
                            )
                    nc.scalar.activation(
                        ps2[:, :W],
                        ps2[:, :W],
                        AF.Exp,
                        scale=exp_scale,
                        accum_out=SUMP[:, sub * cfg.n_vp + vp : sub * cfg.n_vp + vp + 1],
                    )

        # ---- phase C: reduce partials, write outputs ----
        with nc.named_scope("phaseC"):
            for sub in range(cfg.n_sub):
                nc.vector.reduce_sum(
                    SOUT[:, sub : sub + 1],
                    SUMP[:, sub * cfg.n_vp : (sub + 1) * cfg.n_vp],
                    axis=mybir.AxisListType.X,
                )
            nc.sync.dma_start(sumexp_out[:, :], SOUT[:, :])
            nc.sync.dma_start(tgt_out[:, :], TGT[:, :])

    nc.compile()
    return nc


# ---------------- host side ----------------

T, B, H, E, V = 256, 32, 2048, 1024, 50257
NB, NV = 4, 2          # token-groups x vocab-groups


def _plan(lens):
    """Packed-token geometry from lens. Returns (cnt, NVT, counts, NTA, NTB, VC)."""
    cnt = np.clip(np.asarray(lens, np.int64) - 2, 0, None)  # valid tokens per sample
    NVT = int(cnt.sum())
    G = max(1, math.ceil(NVT / NB))
    counts = [max(0, min(G, NVT - g * G)) for g in range(NB)]
    NTB = max(128, math.ceil(G / 128) * 128)
    NTA = math.ceil(NTB / 512) * 512
    VC = math.ceil(V / NV / 128) * 128
    return cnt, NVT, counts, NTA, NTB, VC


def _shard_inputs(hidden, lens, token, W1, b1, W2):
    bf16 = ml_dtypes.bfloat16
    half = H // 2
    cnt, NVT, counts, NTA, NTB, VC = _plan(lens)
    G = max(1, math.ceil(NVT / NB))

    # packed (t, b) pairs, sample-major
    bs = np.repeat(np.arange(B), cnt)
    ts = np.concatenate([np.arange(c) for c in cnt]) if NVT else np.zeros(0, np.int64)
    ctxp = np.concatenate(
        [hidden[ts, bs, :half], hidden[ts + 2, bs, half:]], axis=1
    )  # [NVT, H]
    ctxT = np.ascontiguousarray(ctxp.T).astype(FP8NP)  # [H, NVT]
    tgt_packed = token[ts + 1, bs]  # [NVT]

    W1T = np.ascontiguousarray(W1.T * W1_SCALE).astype(FP8NP)  # [H, E]
    W2T = np.zeros((E, NV * VC), dtype=FP8NP)
    W2T[:, :V] = (W2.T * W2_SCALE).astype(FP8NP)
    b1c = np.ascontiguousarray(b1.reshape(E, 1)).astype(np.float32)
    ones = np.ones((128, 1), dtype=bf16)

    in_maps = []
    for c in range(NB * NV):
        bg, vg = divmod(c, NV)
        lo = bg * G
        n = counts[bg]
        ctxT_c = np.zeros((H, NTA), dtype=FP8NP)
        ctxT_c[:, :n] = ctxT[:, lo : lo + n]
        w2g = W2[tgt_packed[lo : lo + n], :]  # [n, E] fp32 row gather
        w2gT = np.zeros((E, NTA), dtype=bf16)
        w2gT[:, :n] = w2g.T.astype(bf16)
        in_maps.append(
            dict(
                ctxT=ctxT_c,
                w1t=W1T,
                b1=b1c,
                w2t=np.ascontiguousarray(W2T[:, vg * VC : (vg + 1) * VC]),
                w2tgtT=w2gT,
                ones_in=ones,
            )
        )
    return in_maps, (cnt, NVT, counts, NTA, NTB, VC, tgt_packed)


def _combine(results, plan_info, b2, inv_temp):
    """results: list of 8 dicts with sumexp_out [128, n_sub], tgt_out [1, NTA]."""
    cnt, NVT, counts, NTA, NTB, VC, tgt_packed = plan_info
    G = max(1, math.ceil(NVT / NB))
    it = float(np.asarray(inv_temp).reshape(-1)[0])
    n_pad_v = NV * VC - V  # zero-padded vocab cols, all in the last shard
    b2 = np.asarray(b2, dtype=np.float64)

    total_nll = 0.0
    for bg in range(NB):
        n = counts[bg]
        if n == 0:
            continue
        S = np.zeros(NTB, dtype=np.float64)
        for vg in range(NV):
            r = results[bg * NV + vg]
            se = np.asarray(r["sumexp_out"], dtype=np.float64)  # [128, n_sub]
            S += se.T.reshape(NTB)  # token i = sub*128 + p
            if vg == NV - 1:
                S -= n_pad_v  # exp(0)=1 per padded vocab column
        raw = np.asarray(results[bg * NV]["tgt_out"], dtype=np.float64).reshape(NTA)
        logZ = np.log(S[:n])
        tgt_c = tgt_packed[bg * G : bg * G + n]
        logp_tgt = (raw[:n] + b2[tgt_c]) * it - logZ
        total_nll += -logp_tgt.sum()
    return np.float32(total_nll / max(NVT, 1))


def _run(hidden, lens, token, W1, b1, W2, b2, inv_temp, trace=False, tmpdir=None):
    hidden = np.asarray(hidden, dtype=np.float32)
    lens = np.asarray(lens, dtype=np.int32)
    token = np.asarray(token, dtype=np.int32)
    W1 = np.asarray(W1, dtype=np.float32)
    b1 = np.asarray(b1, dtype=np.float32)
    W2 = np.asarray(W2, dtype=np.float32)
    b2 = np.asarray(b2, dtype=np.float32)
    inv_temp = np.asarray(inv_temp, dtype=np.float32)

    use_b2 = bool(np.any(b2 != 0.0))
    _, _, _, NTA, NTB, VC = _plan(lens)
    cfg = Cfg(H, E, NTA, NTB, VC, inv_temp=float(inv_temp.reshape(-1)[0]),
              use_b2=use_b2)
    nc = build_lm_program(cfg)
    in_maps, plan_info = _shard_inputs(hidden, lens, token, W1, b1, W2)
    if use_b2:
        b2p = np.zeros((1, NV * VC), dtype=np.float32)
        b2p[0, :V] = b2 * W2_SCALE
        for c in range(NB * NV):
            vg = c % NV
            in_maps[c]["b2row"] = np.ascontiguousarray(
                b2p[:, vg * VC : (vg + 1) * VC]
            )
    kw = {}
    if trace:
        kw = dict(trace=True, tmpdir=tmpdir)
    res = run_bass_kernel_spmd(nc, in_maps, core_ids=list(range(NB * NV)), **kw)
    return _combine(res.results, plan_info, b2, inv_temp), res


def kernel(hidden, lens, token, W1, b1, W2, b2, inv_temp):
    out, _ = _run(hidden, lens, token, W1, b1, W2, b2, inv_temp)
    return out


# revision 5
# speedup vs baseline: 1.0473x; 1.0473x over previous
"""LMClassifier forward (mean masked cross-entropy) on 8 Trainium2 cores.

Valid-token packing: only positions t < lens[b]-2 contribute to the mean
NLL, so the host packs just those ~48% of (t, b) pairs into dense token
columns before sharding.  Sharding: 8 token-groups, full vocab per core
(no matmul1 replication).  Each core computes
  emb = sigmoid(ctx @ W1.T + b1)             (its packed tokens, all E)
  sumexp[tok] = sum_v exp((emb @ W2.T) * inv_temp)   (full padded vocab)
and ships emb (fp8) back.  Host computes the target logit from emb and
W2 rows (tiny) and assembles the mean NLL over valid tokens; logits are
O(1) so no max-subtraction is needed in the softmax denominator.
"""

import contextlib
import math

import numpy as np
import ml_dtypes

import concourse.bacc as bacc
import concourse.tile as tile
import concourse.mybir as mybir
from concourse.bass_utils import run_bass_kernel_spmd

BF16 = mybir.dt.bfloat16
FP32 = mybir.dt.float32
AF = mybir.ActivationFunctionType


FP8 = mybir.dt.float8e4
FP8NP = mybir.dt.np(mybir.dt.float8e4)
W2_SCALE = 64.0  # keeps fp8-cast W2 out of the denormal range
W1_SCALE = 64.0  # same for W1; sigmoid's free affine divides it back out


class Cfg:
    def __init__(self, H, E, NTA, NTB, VC, inv_temp=1.0, use_b2=False):
        assert H % 512 == 0 and E % 256 == 0
        assert NTA % 512 == 0 and NTB % 128 == 0 and NTB <= NTA
        assert VC % 128 == 0
        self.H, self.E, self.NTA, self.NTB, self.VC = H, E, NTA, NTB, VC
        self.inv_temp = float(inv_temp)
        self.use_b2 = use_b2
        self.n_k = H // 128     # contraction tiles for matmul1
        self.n_e = E // 128     # e-blocks (also contraction tiles for matmul2)
        self.n_s = NTA // 512   # token superblocks (phase A)
        self.n_sub = NTB // 128 # token subblocks (phase B)
        # vocab blocks: 2048-wide plus one optional 128*k tail
        self.vblk = [2048] * (VC // 2048)
        if VC % 2048:
            self.vblk.append(VC % 2048)
        self.n_vp = len(self.vblk)


def build_lm_program(cfg):
    """Build the per-core SPMD Bass program. Returns compiled nc."""
    H, E, NTA, NTB, VC = cfg.H, cfg.E, cfg.NTA, cfg.NTB, cfg.VC
    nc = bacc.Bacc("TRN2", debug=False, target_bir_lowering=False)

    ctxT = nc.dram_tensor("ctxT", [H, NTA], FP8, kind="ExternalInput").ap()
    w1t = nc.dram_tensor("w1t", [H, E], FP8, kind="ExternalInput").ap()
    b1 = nc.dram_tensor("b1", [E, 1], FP32, kind="ExternalInput").ap()
    w2t = nc.dram_tensor("w2t", [E, VC], FP8, kind="ExternalInput").ap()
    if cfg.use_b2:
        b2row = nc.dram_tensor("b2row", [1, VC], FP32, kind="ExternalInput").ap()
    sumexp_out = nc.dram_tensor(
        "sumexp_out", [128, cfg.n_sub], FP32, kind="ExternalOutput"
    ).ap()
    emb8_out = nc.dram_tensor(
        "emb8_out", [128, cfg.n_e, NTA], FP8, kind="ExternalOutput"
    ).ap()

    voff = np.cumsum([0] + cfg.vblk)  # vocab block column offsets
    NKG = 4                           # k-tile groups for W1/ctx streaming
    kg = cfg.n_k // NKG

    with contextlib.ExitStack() as ex:
        tc = ex.enter_context(tile.TileContext(nc))
        qs = [nc.sync, nc.scalar, nc.gpsimd]  # DMA issue queues
        # persistent sbuf tensors
        const_pool = ex.enter_context(tc.tile_pool(name="const", bufs=1))
        w1_pool = ex.enter_context(tc.tile_pool(name="w1", bufs=1))
        emb_pool = ex.enter_context(tc.tile_pool(name="emb", bufs=1))
        acc_pool = ex.enter_context(tc.tile_pool(name="acc", bufs=1))
        # streamed tiles
        ctx_pool = ex.enter_context(tc.tile_pool(name="ctx", bufs=2))
        w2_pool = ex.enter_context(tc.tile_pool(name="w2", bufs=2))

        # ---- constants / startup DMA (4 queues, first-needed first) ----
        W1G = [w1_pool.tile([128, kg, E], FP8, tag=f"w1g{g}", name=f"w1g{g}") for g in range(NKG)]
        CTX0 = [ctx_pool.tile([128, kg, 512], FP8, tag=f"ctx{g}", name=f"ctx{g}") for g in range(NKG)]
        qi = 0
        for g in range(NKG):
            for j in range(kg):
                k = g * kg + j
                qs[qi % len(qs)].dma_start(
                    W1G[g][:, j : j + 1, :], w1t[k * 128 : (k + 1) * 128, :]
                )
                qi += 1
            for j in range(kg):
                k = g * kg + j
                qs[qi % len(qs)].dma_start(
                    CTX0[g][:, j : j + 1, :],
                    ctxT[k * 128 : (k + 1) * 128, 0:512],
                )
                qi += 1
        B1S = const_pool.tile([128, cfg.n_e], FP32, tag="b1s")
        nc.sync.dma_start(B1S[:, :], b1.rearrange("(e p) one -> p (e one)", p=128))
        if cfg.use_b2:
            B2S = const_pool.tile([1, VC], FP32, tag="b2s")
            nc.scalar.dma_start(B2S[:, :], b2row[:, :])
            ONE1 = const_pool.tile([1, 128], FP32, tag="one1")
            nc.any.memset(ONE1[:, :], 1.0)

        # emb8, split by ep-pair so phase B can start as soon as e0/e1 land
        EMB8T = [
            emb_pool.tile([128, 2, NTA], FP8, tag=f"emb8_{j}", name=f"emb8_{j}")
            for j in range(cfg.n_e // 2)
        ]
        SUMP = acc_pool.tile([128, cfg.n_sub * cfg.n_vp], FP32, tag="sump")
        SOUT = acc_pool.tile([128, cfg.n_sub], FP32, tag="sout")

        # prefetch first W2 block under phase A
        W2P0 = w2_pool.tile([128, cfg.n_e, 2048], FP8, tag="w2s")
        for e in range(cfg.n_e):
            qs[e % len(qs)].dma_start(
                W2P0[:, e : e + 1, : cfg.vblk[0]],
                w2t[e * 128 : (e + 1) * 128, voff[0] : voff[1]],
            )

        # ---- phase A: emb = sigmoid(W1 @ ctx + b1), [e, t] layout ----
        sig_scale = 1.0 / W1_SCALE
        with nc.named_scope("phaseA"):
            with tc.tile_pool(name="ps1", bufs=2, space="PSUM") as ps1_pool:
                for s in range(cfg.n_s):
                    if s == 0:
                        CTXS = CTX0
                    else:
                        CTXS = [
                            ctx_pool.tile([128, kg, 512], FP8, tag=f"ctx{g}", name=f"ctx{g}")
                            for g in range(NKG)
                        ]
                        for g in range(NKG):
                            for j in range(kg):
                                k = g * kg + j
                                qs[(g * kg + j) % len(qs)].dma_start(
                                    CTXS[g][:, j : j + 1, :],
                                    ctxT[
                                        k * 128 : (k + 1) * 128,
                                        s * 512 : (s + 1) * 512,
                                    ],
                                )
                    for e in range(cfg.n_e):
                        ps1 = ps1_pool.tile([128, 512], FP32, tag="ps1")
                        for kp in range(cfg.n_k // 2):
                            g, jj = (2 * kp) // kg, (2 * kp) % kg
                            nc.tensor.matmul(
                                ps1[:, :],
                                W1G[g][:, jj : jj + 2, e * 128 : (e + 1) * 128],
                                CTXS[g][:, jj : jj + 2, :],
                                start=(kp == 0),
                                stop=(kp == cfg.n_k // 2 - 1),
                                perf_mode=mybir.MatmulPerfMode.DoubleRow,
                            )
                        nc.scalar.activation(
                            EMB8T[e // 2][:, e % 2 : e % 2 + 1, s * 512 : (s + 1) * 512],
                            ps1[:, :],
                            AF.Sigmoid,
                            bias=B1S[:, e : e + 1],
                            scale=sig_scale,
                        )
                # ship emb back for the host-side target-logit dot
                for j in range(cfg.n_e // 2):
                    qs[j % len(qs)].dma_start(emb8_out[:, 2 * j : 2 * j + 2, :], EMB8T[j][:, :, :])

        # ---- phase B: logits, exp, accumulate ----
        exp_scale = cfg.inv_temp / W2_SCALE
        with nc.named_scope("phaseB"):
            with tc.tile_pool(name="ps2", bufs=2, space="PSUM") as ps2_pool:
                for vp in range(cfg.n_vp):
                    W = cfg.vblk[vp]
                    if vp == 0:
                        W2S8 = W2P0
                    else:
                        W2S8 = w2_pool.tile([128, cfg.n_e, 2048], FP8, tag="w2s")
                        for e in range(cfg.n_e):
                            qs[e % len(qs)].dma_start(
                                W2S8[:, e : e + 1, :W],
                                w2t[e * 128 : (e + 1) * 128, voff[vp] : voff[vp + 1]],
                            )
                    hblk = []
                    h0 = 0
                    while h0 < W:
                        hblk.append((h0, min(512, W - h0)))
                        h0 += 512
                    for sub in range(cfg.n_sub):
                        ps2 = ps2_pool.tile([128, 2048], FP32, tag="ps2")
                        for ep in range(cfg.n_e // 2):
                            lhsT = EMB8T[ep][:, :, sub * 128 : (sub + 1) * 128]
                            for h0, hw in hblk:
                                nc.tensor.matmul(
                                    ps2[:, h0 : h0 + hw],
                                    lhsT,
                                    W2S8[:, 2 * ep : 2 * ep + 2, h0 : h0 + hw],
                                    start=(ep == 0),
                                    stop=(ep == cfg.n_e // 2 - 1) and not cfg.use_b2,
                                    perf_mode=mybir.MatmulPerfMode.DoubleRow,
                                )
                        if cfg.use_b2:
                            for h0, hw in hblk:
                                nc.tensor.matmul(
                                    ps2[:, h0 : h0 + hw],
                                    ONE1[:, :],
                                    B2S[:, voff[vp] + h0 : voff[vp] + h0 + hw],
                                    start=False,
                                    stop=True,
                                )
                        nc.scalar.activation(
                            ps2[:, :W],
                            ps2[:, :W],
                            AF.Exp,
                            scale=exp_scale,
                            accum_out=SUMP[
                                :, sub * cfg.n_vp + vp : sub * cfg.n_vp + vp + 1
                            ],
                        )

        # ---- phase C: reduce partials, write outputs ----
        with nc.named_scope("phaseC"):
            for sub in range(cfg.n_sub):
                nc.vector.reduce_sum(
                    SOUT[:, sub : sub + 1],
                    SUMP[:, sub * cfg.n_vp : (sub + 1) * cfg.n_vp],
                    axis=mybir.AxisListType.X,
                )
            nc.sync.dma_start(sumexp_out[:, :], SOUT[:, :])

    nc.compile()
    return nc


# ---------------- host side ----------------

T, B, H, E, V = 256, 32, 2048, 1024, 50257
NB = 8                 # token-groups; full vocab per core


def _plan(lens):
    cnt = np.clip(np.asarray(lens, np.int64) - 2, 0, None)  # valid tokens per sample
    NVT = int(cnt.sum())
    G = max(1, math.ceil(NVT / NB))
    counts = [max(0, min(G, NVT - g * G)) for g in range(NB)]
    NTB = max(128, math.ceil(G / 128) * 128)
    NTA = math.ceil(NTB / 512) * 512
    VC = math.ceil(V / 128) * 128
    return cnt, NVT, counts, NTA, NTB, VC


def _shard_inputs(hidden, lens, token, W1, b1, W2):
    half = H // 2
    cnt, NVT, counts, NTA, NTB, VC = _plan(lens)
    G = max(1, math.ceil(NVT / NB))

    # packed (t, b) pairs, sample-major
    bs = np.repeat(np.arange(B), cnt)
    ts = np.concatenate([np.arange(c) for c in cnt]) if NVT else np.zeros(0, np.int64)
    ctxp = np.concatenate(
        [hidden[ts, bs, :half], hidden[ts + 2, bs, half:]], axis=1
    )  # [NVT, H]
    ctxT = np.ascontiguousarray(ctxp.T).astype(FP8NP)  # [H, NVT]
    tgt_packed = token[ts + 1, bs]  # [NVT]

    W1T = np.ascontiguousarray(W1.T * W1_SCALE).astype(FP8NP)  # [H, E]
    W2T = np.zeros((E, VC), dtype=FP8NP)
    W2T[:, :V] = (W2.T * W2_SCALE).astype(FP8NP)
    b1c = np.ascontiguousarray(b1.reshape(E, 1)).astype(np.float32)

    in_maps = []
    for g in range(NB):
        lo = g * G
        n = counts[g]
        ctxT_c = np.zeros((H, NTA), dtype=FP8NP)
        ctxT_c[:, :n] = ctxT[:, lo : lo + n]
        in_maps.append(dict(ctxT=ctxT_c, w1t=W1T, b1=b1c, w2t=W2T))
    return in_maps, (cnt, NVT, counts, NTA, NTB, VC, tgt_packed)


def _combine(results, plan_info, W2, b2, inv_temp):
    """results: list of NB dicts with sumexp_out [128, n_sub], emb8_out."""
    cnt, NVT, counts, NTA, NTB, VC, tgt_packed = plan_info
    G = max(1, math.ceil(NVT / NB))
    it = float(np.asarray(inv_temp).reshape(-1)[0])
    n_pad_v = VC - V  # zero-padded vocab cols -> exp(0)=1 each
    b2 = np.asarray(b2, dtype=np.float64)

    total_nll = 0.0
    for g in range(NB):
        n = counts[g]
        if n == 0:
            continue
        r = results[g]
        se = np.asarray(r["sumexp_out"], dtype=np.float64)  # [128, n_sub]
        S = se.T.reshape(NTB)[:n] - n_pad_v  # token i = sub*128 + p
        emb = (
            np.asarray(r["emb8_out"])
            .astype(np.float32)
            .transpose(2, 1, 0)
            .reshape(NTA, E)[:n]
        )  # [n, E]
        tgt_c = tgt_packed[g * G : g * G + n]
        raw = np.einsum("te,te->t", emb, W2[tgt_c, :].astype(np.float32))
        logZ = np.log(S)
        logp_tgt = (raw.astype(np.float64) + b2[tgt_c]) * it - logZ
        total_nll += -logp_tgt.sum()
    return np.float32(total_nll / max(NVT, 1))


def _run(hidden, lens, token, W1, b1, W2, b2, inv_temp, trace=False, tmpdir=None):
    hidden = np.asarray(hidden, dtype=np.float32)
    lens = np.asarray(lens, dtype=np.int32)
    token = np.asarray(token, dtype=np.int32)
    W1 = np.asarray(W1, dtype=np.float32)
    b1 = np.asarray(b1, dtype=np.float32)
    W2 = np.asarray(W2, dtype=np.float32)
    b2 = np.asarray(b2, dtype=np.float32)
    inv_temp = np.asarray(inv_temp, dtype=np.float32)

    use_b2 = bool(np.any(b2 != 0.0))
    _, _, _, NTA, NTB, VC = _plan(lens)
    cfg = Cfg(H, E, NTA, NTB, VC, inv_temp=float(inv_temp.reshape(-1)[0]),
              use_b2=use_b2)
    nc = build_lm_program(cfg)
    in_maps, plan_info = _shard_inputs(hidden, lens, token, W1, b1, W2)
    if use_b2:
        b2p = np.zeros((1, VC), dtype=np.float32)
        b2p[0, :V] = b2 * W2_SCALE
        for g in range(NB):
            in_maps[g]["b2row"] = b2p
    kw = {}
    if trace:
        kw = dict(trace=True, tmpdir=tmpdir)
    res = run_bass_kernel_spmd(nc, in_maps, core_ids=list(range(NB)), **kw)
    return _combine(res.results, plan_info, W2, b2, inv_temp), res


def kernel(hidden, lens, token, W1, b1, W2, b2, inv_temp):
    out, _ = _run(hidden, lens, token, W1, b1, W2, b2, inv_temp)
    return out


# revision 6
# speedup vs baseline: 1.0695x; 1.0213x over previous
"""LMClassifier forward (mean masked cross-entropy) on 8 Trainium2 cores.

Valid-token packing: only positions t < lens[b]-2 contribute to the mean
NLL, so the host packs just those ~48% of (t, b) pairs into dense token
columns before sharding.  Sharding: 8 token-groups, full vocab per core
(no matmul1 replication).  Each core computes
  emb = sigmoid(ctx @ W1.T + b1)             (its packed tokens, all E)
  sumexp[tok] = sum_v exp((emb @ W2.T) * inv_temp)   (full padded vocab)
and ships emb (fp8) back.  Host computes the target logit from emb and
W2 rows (tiny) and assembles the mean NLL over valid tokens; logits are
O(1) so no max-subtraction is needed in the softmax denominator.
"""

import contextlib
import math

import numpy as np
import ml_dtypes

import concourse.bacc as bacc
import concourse.tile as tile
import concourse.mybir as mybir
from concourse.bass_utils import run_bass_kernel_spmd

BF16 = mybir.dt.bfloat16
FP32 = mybir.dt.float32
AF = mybir.ActivationFunctionType


FP8 = mybir.dt.float8e4
FP8NP = mybir.dt.np(mybir.dt.float8e4)
W2_SCALE = 64.0  # keeps fp8-cast W2 out of the denormal range
W1_SCALE = 64.0  # same for W1; sigmoid's free affine divides it back out


class Cfg:
    def __init__(self, H, E, NTA, NTB, VC, inv_temp=1.0, use_b2=False):
        assert H % 512 == 0 and E % 256 == 0
        assert NTA % 512 == 0 and NTB % 128 == 0 and NTB <= NTA
        assert VC % 128 == 0
        self.H, self.E, self.NTA, self.NTB, self.VC = H, E, NTA, NTB, VC
        self.inv_temp = float(inv_temp)
        self.use_b2 = use_b2
        self.n_k = H // 128     # contraction tiles for matmul1
        self.n_e = E // 128     # e-blocks (also contraction tiles for matmul2)
        self.n_s = NTA // 512   # token superblocks (phase A)
        self.n_sub = NTB // 128 # token subblocks (phase B)
        # vocab blocks: 2048-wide plus one optional 128*k tail
        self.vblk = [2048] * (VC // 2048)
        if VC % 2048:
            self.vblk.append(VC % 2048)
        self.n_vp = len(self.vblk)


def build_lm_program(cfg):
    """Build the per-core SPMD Bass program. Returns compiled nc."""
    H, E, NTA, NTB, VC = cfg.H, cfg.E, cfg.NTA, cfg.NTB, cfg.VC
    nc = bacc.Bacc("TRN2", debug=False, target_bir_lowering=False)

    ctxT = nc.dram_tensor("ctxT", [H, NTA], FP8, kind="ExternalInput").ap()
    w1t = nc.dram_tensor("w1t", [H, E], FP8, kind="ExternalInput").ap()
    b1 = nc.dram_tensor("b1", [E, 1], FP32, kind="ExternalInput").ap()
    w2t = nc.dram_tensor("w2t", [E, VC], FP8, kind="ExternalInput").ap()
    if cfg.use_b2:
        b2row = nc.dram_tensor("b2row", [1, VC], FP32, kind="ExternalInput").ap()
    sumexp_out = nc.dram_tensor(
        "sumexp_out", [128, cfg.n_sub], FP32, kind="ExternalOutput"
    ).ap()
    emb8_out = nc.dram_tensor(
        "emb8_out", [128, cfg.n_e, NTA], FP8, kind="ExternalOutput"
    ).ap()

    voff = np.cumsum([0] + cfg.vblk)  # vocab block column offsets
    NKG = 4                           # k-tile groups for W1/ctx streaming
    kg = cfg.n_k // NKG

    with contextlib.ExitStack() as ex:
        tc = ex.enter_context(tile.TileContext(nc))
        qs = [nc.sync, nc.scalar, nc.gpsimd]  # DMA issue queues
        # persistent sbuf tensors
        const_pool = ex.enter_context(tc.tile_pool(name="const", bufs=1))
        w1_pool = ex.enter_context(tc.tile_pool(name="w1", bufs=1))
        emb_pool = ex.enter_context(tc.tile_pool(name="emb", bufs=1))
        acc_pool = ex.enter_context(tc.tile_pool(name="acc", bufs=1))
        # streamed tiles
        ctx_pool = ex.enter_context(tc.tile_pool(name="ctx", bufs=2))
        w2_pool = ex.enter_context(tc.tile_pool(name="w2", bufs=3))

        # ---- constants / startup DMA (4 queues, first-needed first) ----
        W1G = [w1_pool.tile([128, kg, E], FP8, tag=f"w1g{g}", name=f"w1g{g}") for g in range(NKG)]
        CTX0 = [ctx_pool.tile([128, kg, 512], FP8, tag=f"ctx{g}", name=f"ctx{g}") for g in range(NKG)]
        qi = 0
        for g in range(NKG):
            for j in range(kg):
                k = g * kg + j
                qs[qi % len(qs)].dma_start(
                    W1G[g][:, j : j + 1, :], w1t[k * 128 : (k + 1) * 128, :]
                )
                qi += 1
            for j in range(kg):
                k = g * kg + j
                qs[qi % len(qs)].dma_start(
                    CTX0[g][:, j : j + 1, :],
                    ctxT[k * 128 : (k + 1) * 128, 0:512],
                )
                qi += 1
        B1S = const_pool.tile([128, cfg.n_e], FP32, tag="b1s")
        nc.sync.dma_start(B1S[:, :], b1.rearrange("(e p) one -> p (e one)", p=128))
        if cfg.use_b2:
            B2S = const_pool.tile([1, VC], FP32, tag="b2s")
            nc.scalar.dma_start(B2S[:, :], b2row[:, :])
            ONE1 = const_pool.tile([1, 128], FP32, tag="one1")
            nc.any.memset(ONE1[:, :], 1.0)

        # emb8, split by ep-pair so phase B can start as soon as e0/e1 land
        EMB8T = [
            emb_pool.tile([128, 2, NTA], FP8, tag=f"emb8_{j}", name=f"emb8_{j}")
            for j in range(cfg.n_e // 2)
        ]
        SUMP = acc_pool.tile([128, cfg.n_sub * cfg.n_vp], FP32, tag="sump")
        SOUT = acc_pool.tile([128, cfg.n_sub], FP32, tag="sout")

        # prefetch first two W2 blocks under phase A
        w2_prefetch = {}
        for vp in range(min(2, cfg.n_vp)):
            W2P = w2_pool.tile([128, cfg.n_e, 2048], FP8, tag="w2s", name="w2p")
            for e in range(cfg.n_e):
                qs[e % len(qs)].dma_start(
                    W2P[:, e : e + 1, : cfg.vblk[vp]],
                    w2t[e * 128 : (e + 1) * 128, voff[vp] : voff[vp + 1]],
                )
            w2_prefetch[vp] = W2P

        # ---- phase A: emb = sigmoid(W1 @ ctx + b1), [e, t] layout ----
        sig_scale = 1.0 / W1_SCALE
        with nc.named_scope("phaseA"):
            with tc.tile_pool(name="ps1", bufs=2, space="PSUM") as ps1_pool:
                for s in range(cfg.n_s):
                    if s == 0:
                        CTXS = CTX0
                    else:
                        CTXS = [
                            ctx_pool.tile([128, kg, 512], FP8, tag=f"ctx{g}", name=f"ctx{g}")
                            for g in range(NKG)
                        ]
                        for g in range(NKG):
                            for j in range(kg):
                                k = g * kg + j
                                qs[(g * kg + j) % len(qs)].dma_start(
                                    CTXS[g][:, j : j + 1, :],
                                    ctxT[
                                        k * 128 : (k + 1) * 128,
                                        s * 512 : (s + 1) * 512,
                                    ],
                                )
                    for e in range(cfg.n_e):
                        ps1 = ps1_pool.tile([128, 512], FP32, tag="ps1")
                        for kp in range(cfg.n_k // 2):
                            g, jj = (2 * kp) // kg, (2 * kp) % kg
                            nc.tensor.matmul(
                                ps1[:, :],
                                W1G[g][:, jj : jj + 2, e * 128 : (e + 1) * 128],
                                CTXS[g][:, jj : jj + 2, :],
                                start=(kp == 0),
                                stop=(kp == cfg.n_k // 2 - 1),
                                perf_mode=mybir.MatmulPerfMode.DoubleRow,
                            )
                        nc.scalar.activation(
                            EMB8T[e // 2][:, e % 2 : e % 2 + 1, s * 512 : (s + 1) * 512],
                            ps1[:, :],
                            AF.Sigmoid,
                            bias=B1S[:, e : e + 1],
                            scale=sig_scale,
                        )
                # ship emb back for the host-side target-logit dot
                for j in range(cfg.n_e // 2):
                    qs[j % len(qs)].dma_start(emb8_out[:, 2 * j : 2 * j + 2, :], EMB8T[j][:, :, :])

        # ---- phase B: logits, exp, accumulate ----
        exp_scale = cfg.inv_temp / W2_SCALE
        with nc.named_scope("phaseB"):
            with tc.tile_pool(name="ps2", bufs=2, space="PSUM") as ps2_pool:
                for vp in range(cfg.n_vp):
                    W = cfg.vblk[vp]
                    if vp in w2_prefetch:
                        W2S8 = w2_prefetch.pop(vp)
                    else:
                        W2S8 = w2_pool.tile([128, cfg.n_e, 2048], FP8, tag="w2s")
                        for e in range(cfg.n_e):
                            qs[e % len(qs)].dma_start(
                                W2S8[:, e : e + 1, :W],
                                w2t[e * 128 : (e + 1) * 128, voff[vp] : voff[vp + 1]],
                            )
                    hblk = []
                    h0 = 0
                    while h0 < W:
                        hblk.append((h0, min(512, W - h0)))
                        h0 += 512
                    for sub in range(cfg.n_sub):
                        ps2 = ps2_pool.tile([128, 2048], FP32, tag="ps2")
                        for ep in range(cfg.n_e // 2):
                            lhsT = EMB8T[ep][:, :, sub * 128 : (sub + 1) * 128]
                            for h0, hw in hblk:
                                nc.tensor.matmul(
                                    ps2[:, h0 : h0 + hw],
                                    lhsT,
                                    W2S8[:, 2 * ep : 2 * ep + 2, h0 : h0 + hw],
                                    start=(ep == 0),
                                    stop=(ep == cfg.n_e // 2 - 1) and not cfg.use_b2,
                                    perf_mode=mybir.MatmulPerfMode.DoubleRow,
                                )
                        if cfg.use_b2:
                            for h0, hw in hblk:
                                nc.tensor.matmul(
                                    ps2[:, h0 : h0 + hw],
                                    ONE1[:, :],
                                    B2S[:, voff[vp] + h0 : voff[vp] + h0 + hw],
                                    start=False,
                                    stop=True,
                                )
                        nc.scalar.activation(
                            ps2[:, :W],
                            ps2[:, :W],
                            AF.Exp,
                            scale=exp_scale,
                            accum_out=SUMP[
                                :, sub * cfg.n_vp + vp : sub * cfg.n_vp + vp + 1
                            ],
                        )

        # ---- phase C: reduce partials, write outputs ----
        with nc.named_scope("phaseC"):
            for sub in range(cfg.n_sub):
                nc.vector.reduce_sum(
                    SOUT[:, sub : sub + 1],
                    SUMP[:, sub * cfg.n_vp : (sub + 1) * cfg.n_vp],
                    axis=mybir.AxisListType.X,
                )
            nc.sync.dma_start(sumexp_out[:, :], SOUT[:, :])

    nc.compile()
    return nc


# ---------------- host side ----------------

T, B, H, E, V = 256, 32, 2048, 1024, 50257
NB = 8                 # token-groups; full vocab per core


def _plan(lens):
    cnt = np.clip(np.asarray(lens, np.int64) - 2, 0, None)  # valid tokens per sample
    NVT = int(cnt.sum())
    G = max(1, math.ceil(NVT / NB))
    counts = [max(0, min(G, NVT - g * G)) for g in range(NB)]
    NTB = max(128, math.ceil(G / 128) * 128)
    NTA = math.ceil(NTB / 512) * 512
    VC = math.ceil(V / 128) * 128
    return cnt, NVT, counts, NTA, NTB, VC


def _shard_inputs(hidden, lens, token, W1, b1, W2):
    half = H // 2
    cnt, NVT, counts, NTA, NTB, VC = _plan(lens)
    G = max(1, math.ceil(NVT / NB))

    # packed (t, b) pairs, sample-major
    bs = np.repeat(np.arange(B), cnt)
    ts = np.concatenate([np.arange(c) for c in cnt]) if NVT else np.zeros(0, np.int64)
    ctxp = np.concatenate(
        [hidden[ts, bs, :half], hidden[ts + 2, bs, half:]], axis=1
    )  # [NVT, H]
    ctxT = np.ascontiguousarray(ctxp.T).astype(FP8NP)  # [H, NVT]
    tgt_packed = token[ts + 1, bs]  # [NVT]

    W1T = np.ascontiguousarray(W1.T * W1_SCALE).astype(FP8NP)  # [H, E]
    W2T = np.zeros((E, VC), dtype=FP8NP)
    W2T[:, :V] = (W2.T * W2_SCALE).astype(FP8NP)
    b1c = np.ascontiguousarray(b1.reshape(E, 1)).astype(np.float32)

    in_maps = []
    for g in range(NB):
        lo = g * G
        n = counts[g]
        ctxT_c = np.zeros((H, NTA), dtype=FP8NP)
        ctxT_c[:, :n] = ctxT[:, lo : lo + n]
        in_maps.append(dict(ctxT=ctxT_c, w1t=W1T, b1=b1c, w2t=W2T))
    return in_maps, (cnt, NVT, counts, NTA, NTB, VC, tgt_packed)


def _combine(results, plan_info, W2, b2, inv_temp):
    """results: list of NB dicts with sumexp_out [128, n_sub], emb8_out."""
    cnt, NVT, counts, NTA, NTB, VC, tgt_packed = plan_info
    G = max(1, math.ceil(NVT / NB))
    it = float(np.asarray(inv_temp).reshape(-1)[0])
    n_pad_v = VC - V  # zero-padded vocab cols -> exp(0)=1 each
    b2 = np.asarray(b2, dtype=np.float64)

    total_nll = 0.0
    for g in range(NB):
        n = counts[g]
        if n == 0:
            continue
        r = results[g]
        se = np.asarray(r["sumexp_out"], dtype=np.float64)  # [128, n_sub]
        S = se.T.reshape(NTB)[:n] - n_pad_v  # token i = sub*128 + p
        emb = (
            np.asarray(r["emb8_out"])
            .astype(np.float32)
            .transpose(2, 1, 0)
            .reshape(NTA, E)[:n]
        )  # [n, E]
        tgt_c = tgt_packed[g * G : g * G + n]
        raw = np.einsum("te,te->t", emb, W2[tgt_c, :].astype(np.float32))
        logZ = np.log(S)
        logp_tgt = (raw.astype(np.float64) + b2[tgt_c]) * it - logZ
        total_nll += -logp_tgt.sum()
    return np.float32(total_nll / max(NVT, 1))


def _run(hidden, lens, token, W1, b1, W2, b2, inv_temp, trace=False, tmpdir=None):
    hidden = np.asarray(hidden, dtype=np.float32)
    lens = np.asarray(lens, dtype=np.int32)
    token = np.asarray(token, dtype=np.int32)
    W1 = np.asarray(W1, dtype=np.float32)
    b1 = np.asarray(b1, dtype=np.float32)
    W2 = np.asarray(W2, dtype=np.float32)
    b2 = np.asarray(b2, dtype=np.float32)
    inv_temp = np.asarray(inv_temp, dtype=np.float32)

    use_b2 = bool(np.any(b2 != 0.0))
    _, _, _, NTA, NTB, VC = _plan(lens)
    cfg = Cfg(H, E, NTA, NTB, VC, inv_temp=float(inv_temp.reshape(-1)[0]),
              use_b2=use_b2)
    nc = build_lm_program(cfg)
    in_maps, plan_info = _shard_inputs(hidden, lens, token, W1, b1, W2)
    if use_b2:
        b2p = np.zeros((1, VC), dtype=np.float32)
        b2p[0, :V] = b2 * W2_SCALE
        for g in range(NB):
            in_maps[g]["b2row"] = b2p
    kw = {}
    if trace:
        kw = dict(trace=True, tmpdir=tmpdir)
    res = run_bass_kernel_spmd(nc, in_maps, core_ids=list(range(NB)), **kw)
    return _combine(res.results, plan_info, W2, b2, inv_temp), res


def kernel(hidden, lens, token, W1, b1, W2, b2, inv_temp):
    out, _ = _run(hidden, lens, token, W1, b1, W2, b2, inv_temp)
    return out


# revision 7
# speedup vs baseline: 1.2907x; 1.2068x over previous
"""LMClassifier forward (mean masked cross-entropy) on 8 Trainium2 cores.

Valid-token packing: only positions t < lens[b]-2 contribute to the mean
NLL, so the host packs just those ~48% of (t, b) pairs into dense token
columns before sharding.  Sharding: 8 token-groups, full vocab per core
(no matmul1 replication).  Each core computes
  emb = sigmoid(ctx @ W1.T + b1)             (its packed tokens, all E)
  sumexp[tok] = sum_v exp((emb @ W2.T) * inv_temp)   (full padded vocab)
and ships emb (fp8) back.  Host computes the target logit from emb and
W2 rows (tiny) and assembles the mean NLL over valid tokens; logits are
O(1) so no max-subtraction is needed in the softmax denominator.
"""

import contextlib
import math

import numpy as np
import ml_dtypes

import concourse.bacc as bacc
import concourse.tile as tile
import concourse.mybir as mybir
from concourse.bass_utils import run_bass_kernel_spmd

BF16 = mybir.dt.bfloat16
FP32 = mybir.dt.float32
AF = mybir.ActivationFunctionType


FP8 = mybir.dt.float8e4
FP8NP = mybir.dt.np(mybir.dt.float8e4)
W2_SCALE = 64.0  # keeps fp8-cast W2 out of the denormal range
W1_SCALE = 64.0  # same for W1; sigmoid's free affine divides it back out


class Cfg:
    def __init__(self, H, E, NTA, NTB, VC, inv_temp=1.0, use_b2=False):
        assert H % 512 == 0 and E % 256 == 0
        assert NTA % 512 == 0 and NTB % 128 == 0 and NTB <= NTA
        assert VC % 128 == 0
        self.H, self.E, self.NTA, self.NTB, self.VC = H, E, NTA, NTB, VC
        self.inv_temp = float(inv_temp)
        self.use_b2 = use_b2
        self.n_k = H // 128     # contraction tiles for matmul1
        self.n_e = E // 128     # e-blocks (also contraction tiles for matmul2)
        self.n_s = NTA // 512   # token superblocks (phase A)
        self.n_sub = NTB // 128 # token subblocks (phase B)
        # vocab blocks: 2048-wide plus one optional 128*k tail
        self.vblk = [2048] * (VC // 2048)
        if VC % 2048:
            self.vblk.append(VC % 2048)
        self.n_vp = len(self.vblk)


def build_lm_program(cfg):
    """Build the per-core SPMD Bass program. Returns compiled nc."""
    H, E, NTA, NTB, VC = cfg.H, cfg.E, cfg.NTA, cfg.NTB, cfg.VC
    nc = bacc.Bacc("TRN2", debug=False, target_bir_lowering=False)

    ctxT = nc.dram_tensor("ctxT", [H, NTA], FP8, kind="ExternalInput").ap()
    w1t = nc.dram_tensor("w1t", [H, E], FP8, kind="ExternalInput").ap()
    b1 = nc.dram_tensor("b1", [E, 1], FP32, kind="ExternalInput").ap()
    w2t = nc.dram_tensor("w2t", [E, VC], FP8, kind="ExternalInput").ap()
    if cfg.use_b2:
        b2row = nc.dram_tensor("b2row", [1, VC], FP32, kind="ExternalInput").ap()
    sumexp_out = nc.dram_tensor(
        "sumexp_out", [128, cfg.n_sub], FP32, kind="ExternalOutput"
    ).ap()
    emb8_out = nc.dram_tensor(
        "emb8_out", [128, cfg.n_e, NTA], FP8, kind="ExternalOutput"
    ).ap()

    voff = np.cumsum([0] + cfg.vblk)  # vocab block column offsets
    NKG = 4                           # k-tile groups for W1/ctx streaming
    kg = cfg.n_k // NKG

    with contextlib.ExitStack() as ex:
        tc = ex.enter_context(tile.TileContext(nc))
        qs = [nc.sync, nc.scalar, nc.gpsimd]  # DMA issue queues
        # persistent sbuf tensors
        const_pool = ex.enter_context(tc.tile_pool(name="const", bufs=1))
        w1_pool = ex.enter_context(tc.tile_pool(name="w1", bufs=1))
        emb_pool = ex.enter_context(tc.tile_pool(name="emb", bufs=1))
        acc_pool = ex.enter_context(tc.tile_pool(name="acc", bufs=1))
        # streamed tiles
        ctx_pool = ex.enter_context(tc.tile_pool(name="ctx", bufs=2))
        w2_pool = ex.enter_context(tc.tile_pool(name="w2", bufs=3))

        # ---- constants / startup DMA (batched descriptors, first-needed first;
        # scalar queue stays free for activations) ----
        w1r = w1t.rearrange("(k p) e -> p k e", p=128)
        ctxr = ctxT.rearrange("(k p) t -> p k t", p=128)
        w2r = w2t.rearrange("(e p) v -> p e v", p=128)
        W1G = [w1_pool.tile([128, kg, E], FP8, tag=f"w1g{g}", name=f"w1g{g}") for g in range(NKG)]
        CTX0 = [ctx_pool.tile([128, kg, 512], FP8, tag=f"ctx{g}", name=f"ctx{g}") for g in range(NKG)]
        for g in range(NKG):
            nc.gpsimd.dma_start(W1G[g][:, :, :], w1r[:, g * kg : (g + 1) * kg, :])
            nc.sync.dma_start(CTX0[g][:, :, :], ctxr[:, g * kg : (g + 1) * kg, 0:512])
        B1S = const_pool.tile([128, cfg.n_e], FP32, tag="b1s")
        nc.scalar.dma_start(B1S[:, :], b1.rearrange("(e p) one -> p (e one)", p=128))
        if cfg.use_b2:
            B2S = const_pool.tile([1, VC], FP32, tag="b2s")
            nc.scalar.dma_start(B2S[:, :], b2row[:, :])
            ONE1 = const_pool.tile([1, 128], FP32, tag="one1")
            nc.any.memset(ONE1[:, :], 1.0)

        # emb8, split by ep-pair so phase B can start as soon as e0/e1 land
        EMB8T = [
            emb_pool.tile([128, 2, NTA], FP8, tag=f"emb8_{j}", name=f"emb8_{j}")
            for j in range(cfg.n_e // 2)
        ]
        SUMP = acc_pool.tile([128, cfg.n_sub * cfg.n_vp], FP32, tag="sump")
        SOUT = acc_pool.tile([128, cfg.n_sub], FP32, tag="sout")

        # prefetch first two W2 blocks under phase A
        ne2 = cfg.n_e // 2
        w2_prefetch = {}
        for vp in range(min(2, cfg.n_vp)):
            W2P = w2_pool.tile([128, cfg.n_e, 2048], FP8, tag="w2s", name="w2p")
            nc.gpsimd.dma_start(
                W2P[:, :ne2, : cfg.vblk[vp]],
                w2r[:, :ne2, voff[vp] : voff[vp + 1]],
            )
            nc.sync.dma_start(
                W2P[:, ne2:, : cfg.vblk[vp]],
                w2r[:, ne2:, voff[vp] : voff[vp + 1]],
            )
            w2_prefetch[vp] = W2P

        # ---- phase A: emb = sigmoid(W1 @ ctx + b1), [e, t] layout ----
        sig_scale = 1.0 / W1_SCALE
        with nc.named_scope("phaseA"):
            with tc.tile_pool(name="ps1", bufs=4, space="PSUM") as ps1_pool:
                for s in range(cfg.n_s):
                    if s == 0:
                        CTXS = CTX0
                    else:
                        CTXS = [
                            ctx_pool.tile([128, kg, 512], FP8, tag=f"ctx{g}", name=f"ctx{g}")
                            for g in range(NKG)
                        ]
                        for g in range(NKG):
                            nc.sync.dma_start(
                                CTXS[g][:, :, :],
                                ctxr[:, g * kg : (g + 1) * kg, s * 512 : (s + 1) * 512],
                            )
                    for e in range(cfg.n_e):
                        ps1 = ps1_pool.tile([128, 512], FP32, tag="ps1")
                        for kp in range(cfg.n_k // 2):
                            g, jj = (2 * kp) // kg, (2 * kp) % kg
                            nc.tensor.matmul(
                                ps1[:, :],
                                W1G[g][:, jj : jj + 2, e * 128 : (e + 1) * 128],
                                CTXS[g][:, jj : jj + 2, :],
                                start=(kp == 0),
                                stop=(kp == cfg.n_k // 2 - 1),
                                perf_mode=mybir.MatmulPerfMode.DoubleRow,
                            )
                        nc.scalar.activation(
                            EMB8T[e // 2][:, e % 2 : e % 2 + 1, s * 512 : (s + 1) * 512],
                            ps1[:, :],
                            AF.Sigmoid,
                            bias=B1S[:, e : e + 1],
                            scale=sig_scale,
                        )
                # ship emb back for the host-side target-logit dot
                for j in range(cfg.n_e // 2):
                    nc.gpsimd.dma_start(emb8_out[:, 2 * j : 2 * j + 2, :], EMB8T[j][:, :, :])

        # ---- phase B: logits, exp, accumulate ----
        exp_scale = cfg.inv_temp / W2_SCALE
        with nc.named_scope("phaseB"):
            with tc.tile_pool(name="ps2", bufs=2, space="PSUM") as ps2_pool:
                for vp in range(cfg.n_vp):
                    W = cfg.vblk[vp]
                    if vp in w2_prefetch:
                        W2S8 = w2_prefetch.pop(vp)
                    else:
                        W2S8 = w2_pool.tile([128, cfg.n_e, 2048], FP8, tag="w2s")
                        nc.gpsimd.dma_start(
                            W2S8[:, :ne2, :W], w2r[:, :ne2, voff[vp] : voff[vp + 1]]
                        )
                        nc.sync.dma_start(
                            W2S8[:, ne2:, :W], w2r[:, ne2:, voff[vp] : voff[vp + 1]]
                        )
                    hblk = []
                    h0 = 0
                    while h0 < W:
                        hblk.append((h0, min(512, W - h0)))
                        h0 += 512
                    for sub in range(cfg.n_sub):
                        ps2 = ps2_pool.tile([128, 2048], FP32, tag="ps2")
                        for ep in range(cfg.n_e // 2):
                            lhsT = EMB8T[ep][:, :, sub * 128 : (sub + 1) * 128]
                            for h0, hw in hblk:
                                nc.tensor.matmul(
                                    ps2[:, h0 : h0 + hw],
                                    lhsT,
                                    W2S8[:, 2 * ep : 2 * ep + 2, h0 : h0 + hw],
                                    start=(ep == 0),
                                    stop=(ep == cfg.n_e // 2 - 1) and not cfg.use_b2,
                                    perf_mode=mybir.MatmulPerfMode.DoubleRow,
                                )
                        if cfg.use_b2:
                            for h0, hw in hblk:
                                nc.tensor.matmul(
                                    ps2[:, h0 : h0 + hw],
                                    ONE1[:, :],
                                    B2S[:, voff[vp] + h0 : voff[vp] + h0 + hw],
                                    start=False,
                                    stop=True,
                                )
                        nc.scalar.activation(
                            ps2[:, :W],
                            ps2[:, :W],
                            AF.Exp,
                            scale=exp_scale,
                            accum_out=SUMP[
                                :, sub * cfg.n_vp + vp : sub * cfg.n_vp + vp + 1
                            ],
                        )

        # ---- phase C: reduce partials, write outputs ----
        with nc.named_scope("phaseC"):
            for sub in range(cfg.n_sub):
                nc.vector.reduce_sum(
                    SOUT[:, sub : sub + 1],
                    SUMP[:, sub * cfg.n_vp : (sub + 1) * cfg.n_vp],
                    axis=mybir.AxisListType.X,
                )
            nc.sync.dma_start(sumexp_out[:, :], SOUT[:, :])

    nc.compile()
    return nc


# ---------------- host side ----------------

T, B, H, E, V = 256, 32, 2048, 1024, 50257
NB = 8                 # token-groups; full vocab per core


def _plan(lens):
    cnt = np.clip(np.asarray(lens, np.int64) - 2, 0, None)  # valid tokens per sample
    NVT = int(cnt.sum())
    G = max(1, math.ceil(NVT / NB))
    counts = [max(0, min(G, NVT - g * G)) for g in range(NB)]
    NTB = max(128, math.ceil(G / 128) * 128)
    NTA = math.ceil(NTB / 512) * 512
    VC = math.ceil(V / 128) * 128
    return cnt, NVT, counts, NTA, NTB, VC


def _shard_inputs(hidden, lens, token, W1, b1, W2):
    half = H // 2
    cnt, NVT, counts, NTA, NTB, VC = _plan(lens)
    G = max(1, math.ceil(NVT / NB))

    # packed (t, b) pairs, sample-major
    bs = np.repeat(np.arange(B), cnt)
    ts = np.concatenate([np.arange(c) for c in cnt]) if NVT else np.zeros(0, np.int64)
    ctxp = np.concatenate(
        [hidden[ts, bs, :half], hidden[ts + 2, bs, half:]], axis=1
    )  # [NVT, H]
    ctxT = np.ascontiguousarray(ctxp.T).astype(FP8NP)  # [H, NVT]
    tgt_packed = token[ts + 1, bs]  # [NVT]

    W1T = np.ascontiguousarray(W1.T * W1_SCALE).astype(FP8NP)  # [H, E]
    W2T = np.zeros((E, VC), dtype=FP8NP)
    W2T[:, :V] = (W2.T * W2_SCALE).astype(FP8NP)
    b1c = np.ascontiguousarray(b1.reshape(E, 1)).astype(np.float32)

    in_maps = []
    for g in range(NB):
        lo = g * G
        n = counts[g]
        ctxT_c = np.zeros((H, NTA), dtype=FP8NP)
        ctxT_c[:, :n] = ctxT[:, lo : lo + n]
        in_maps.append(dict(ctxT=ctxT_c, w1t=W1T, b1=b1c, w2t=W2T))
    return in_maps, (cnt, NVT, counts, NTA, NTB, VC, tgt_packed)


def _combine(results, plan_info, W2, b2, inv_temp):
    """results: list of NB dicts with sumexp_out [128, n_sub], emb8_out."""
    cnt, NVT, counts, NTA, NTB, VC, tgt_packed = plan_info
    G = max(1, math.ceil(NVT / NB))
    it = float(np.asarray(inv_temp).reshape(-1)[0])
    n_pad_v = VC - V  # zero-padded vocab cols -> exp(0)=1 each
    b2 = np.asarray(b2, dtype=np.float64)

    total_nll = 0.0
    for g in range(NB):
        n = counts[g]
        if n == 0:
            continue
        r = results[g]
        se = np.asarray(r["sumexp_out"], dtype=np.float64)  # [128, n_sub]
        S = se.T.reshape(NTB)[:n] - n_pad_v  # token i = sub*128 + p
        emb = (
            np.asarray(r["emb8_out"])
            .astype(np.float32)
            .transpose(2, 1, 0)
            .reshape(NTA, E)[:n]
        )  # [n, E]
        tgt_c = tgt_packed[g * G : g * G + n]
        raw = np.einsum("te,te->t", emb, W2[tgt_c, :].astype(np.float32))
        logZ = np.log(S)
        logp_tgt = (raw.astype(np.float64) + b2[tgt_c]) * it - logZ
        total_nll += -logp_tgt.sum()
    return np.float32(total_nll / max(NVT, 1))


def _run(hidden, lens, token, W1, b1, W2, b2, inv_temp, trace=False, tmpdir=None):
    hidden = np.asarray(hidden, dtype=np.float32)
    lens = np.asarray(lens, dtype=np.int32)
    token = np.asarray(token, dtype=np.int32)
    W1 = np.asarray(W1, dtype=np.float32)
    b1 = np.asarray(b1, dtype=np.float32)
    W2 = np.asarray(W2, dtype=np.float32)
    b2 = np.asarray(b2, dtype=np.float32)
    inv_temp = np.asarray(inv_temp, dtype=np.float32)

    use_b2 = bool(np.any(b2 != 0.0))
    _, _, _, NTA, NTB, VC = _plan(lens)
    cfg = Cfg(H, E, NTA, NTB, VC, inv_temp=float(inv_temp.reshape(-1)[0]),
              use_b2=use_b2)
    nc = build_lm_program(cfg)
    in_maps, plan_info = _shard_inputs(hidden, lens, token, W1, b1, W2)
    if use_b2:
        b2p = np.zeros((1, VC), dtype=np.float32)
        b2p[0, :V] = b2 * W2_SCALE
        for g in range(NB):
            in_maps[g]["b2row"] = b2p
    kw = {}
    if trace:
        kw = dict(trace=True, tmpdir=tmpdir)
    res = run_bass_kernel_spmd(nc, in_maps, core_ids=list(range(NB)), **kw)
    return _combine(res.results, plan_info, W2, b2, inv_temp), res


def kernel(hidden, lens, token, W1, b1, W2, b2, inv_temp):
    out, _ = _run(hidden, lens, token, W1, b1, W2, b2, inv_temp)
    return out
